# revision 13
# baseline (speedup 1.0000x reference)
"""BertSelfAttention (disentangled seg-bias variant) on 8 Trainium2 NeuronCores.

Sharding: tensor-parallel over heads (2 heads per core); both batches per core.

Math per (b, h):
  q = hs @ Wq.T + bq ; k = hs @ Wk.T ; v = hs @ Wv.T + bv
  k' = scale*k + seg_rep          (seg folded into K-projection PSUM via a
                                   rank-2 matmul: seg_rep = t0*(1-s) + t1*s)
  scoresT[j,i] = sum_c k'[c,j] q[c,i]            (j on partitions)
  r1[j] = b_q_s . seg_rep[j]  -> per-partition bias of the exp
  probsT = exp(scoresT + r1[j]) * erelT          (erel = exp(rel) on host)
  ctxT[c,i] = sum_j v[j,c] probsT[j,i] ; denom via ones-columns in the same MM
  out = ctxT / denom

Schedule: stage B is 4 units (b,ib), 16 j-tiles each, software-pipelined so
the PE stream per j-tile is [QK(jt), PV(jt-1), fillers]; exp (ACT) and the
erel multiply (DVE 2x fp16) chase one j-tile behind.  All projection work
beyond the minimal warmup (Q pt0-1, K pt0, V pt0 + transposes of batch 0) is
drip-fed into PE slack as "filler" closures, ordered so each piece completes
just ahead of its consumer.
"""

import numpy as np
from contextlib import ExitStack

import concourse.bass as bass
import concourse.bacc as bacc
import concourse.mybir as mybir
import concourse.tile as tile
from concourse.bass_utils import run_bass_kernel_spmd
from concourse.masks import make_identity

B, S, D, H = 2, 2048, 1024, 16
DH = D // H                      # 64
N_CORES = 8
HPC = H // N_CORES               # heads per core = 2
NKC = D // 128                   # contraction chunks = 8
NPT = S // 512                   # 512-wide position tiles = 4
NJT = S // 128                   # 128-wide j tiles = 16
NIB = S // 1024                  # 1024-wide i blocks = 2
SCALE = 1.0 / np.sqrt(DH)        # 0.125, exact in fp16

F32 = mybir.dt.float32
F16 = mybir.dt.float16

REL_PREFETCH = 5                 # erel pair-loads emitted ahead
FILL_RATES = (5, 3, 1, 1)        # filler closures per jt slot, per unit


def emit_body(nc, tc, ctx, pools, aps, use_mask, opts=None):
    opts = opts or {}
    (const, hspool, qpool, kpool, vtpool, vnpool, relpool, probpool,
     pspool, pvpool, denpool, rcpbpool, ctxpool, scrpool) = pools
    hsT, wT, relT, seg2, stab, cpkd, crowd, out = aps

    w_sb = const.tile([128, 3, NKC, 128], F16, tag="w_sb")
    nc.sync.dma_start(out=w_sb, in_=wT.rearrange("p k d c -> d p k c"))
    # fold softmax scale into Wk (0.125 is exact in fp16)
    nc.vector.tensor_scalar_mul(w_sb[:, 1], w_sb[:, 1], SCALE)

    # packed fp32 per-partition consts: col0 bqc, col1 bvc, col2.. segc
    cpk = const.tile([128, 2 + B * NJT], F32, tag="cpk")
    nc.sync.dma_start(out=cpk, in_=cpkd)
    bqc_sb = cpk[:, 0:1]
    bvc_sb = cpk[:, 1:2]
    segc_sb = cpk[:, 2:2 + B * NJT]
    # packed fp32 row consts along the free dim: t0 | t1 | bqs
    crow = const.tile([1, 3 * 128], F32, tag="crow")
    nc.sync.dma_start(out=crow, in_=crowd)
    t0f = crow[0:1, 0:128]
    t1f = crow[0:1, 128:256]
    bqs_sb = crow[0:1, 256:384]

    stab_sb = const.tile([2, 128], F16, tag="stab_sb")
    nc.sync.dma_start(out=stab_sb, in_=stab)
    seg2_sb = const.tile([2, B * S], F16, tag="seg2_sb")
    nc.sync.dma_start(out=seg2_sb, in_=seg2.rearrange("b r s -> r b s"))

    ident = const.tile([128, 128], F16, tag="ident")
    make_identity(nc, ident)

    # --- r1 (b_q_s . seg_rep) per-partition bias columns -------------------
    # gamma0_h = sum_{c in head h} bqs[c]*t0[c]; gamma1_h likewise with t1.
    # r1[j] = gamma0_h + (gamma1_h - gamma0_h) * s_j
    prod = const.tile([1, 128], F32, tag="prod")
    g_row = const.tile([1, 4], F32, tag="g_row")   # [g0_h0, g0_h1, g1_h0, g1_h1]
    b_row = const.tile([1, 4], F32, tag="b_row")   # [d_h0, d_h1, g0_h0, g0_h1]
    ones1 = const.tile([1, 128], F32, tag="ones1")
    nc.vector.memset(ones1, 1.0)
    nc.vector.tensor_mul(prod, bqs_sb, t0f)
    nc.vector.tensor_reduce(g_row[0:1, 0:1], prod[0:1, 0:64],
                            axis=mybir.AxisListType.X, op=mybir.AluOpType.add)
    nc.vector.tensor_reduce(g_row[0:1, 1:2], prod[0:1, 64:128],
                            axis=mybir.AxisListType.X, op=mybir.AluOpType.add)
    nc.vector.tensor_mul(prod, bqs_sb, t1f)
    nc.vector.tensor_reduce(g_row[0:1, 2:3], prod[0:1, 0:64],
                            axis=mybir.AxisListType.X, op=mybir.AluOpType.add)
    nc.vector.tensor_reduce(g_row[0:1, 3:4], prod[0:1, 64:128],
                            axis=mybir.AxisListType.X, op=mybir.AluOpType.add)
    # delta_h = g1_h - g0_h in slots 0:2, g0_h in slots 2:4
    nc.vector.tensor_sub(b_row[0:1, 0:2], g_row[0:1, 2:4], g_row[0:1, 0:2])
    nc.vector.tensor_copy(b_row[0:1, 2:4], g_row[0:1, 0:2])
    # broadcast the 4 values to all 128 partitions via a K=1 matmul.
    # Emitted via closure AFTER the warmup projections so the tiny psB
    # matmul does not sit at the head of the PE stream gating everything
    # on the (late-arriving) const DMAs.
    r1c = const.tile([128, B * HPC * NJT], F32, tag="r1c")

    def emit_r1c():
        psB = pspool.tile([128, 4], F32, tag="ps", name="psB")
        nc.tensor.matmul(psB, lhsT=ones1, rhs=b_row, start=True, stop=True)
        bc4 = const.tile([128, 4], F32, tag="bc4")
        nc.vector.tensor_copy(bc4, psB)
        for b in range(B):
            for hl in range(HPC):
                nc.vector.tensor_scalar(
                    out=r1c[:, (b * HPC + hl) * NJT:(b * HPC + hl + 1) * NJT],
                    in0=segc_sb[:, b * NJT:(b + 1) * NJT],
                    scalar1=bc4[:, hl:hl + 1],
                    scalar2=bc4[:, 2 + hl:2 + hl + 1],
                    op0=mybir.AluOpType.mult,
                    op1=mybir.AluOpType.add,
                )

    # --- projection machinery ---------------------------------------------
    qT, kT, vn = [None] * B, [None] * B, [None] * B

    def emit_hsb(b, defer_dma=False):
        """DMA hidden-states (token-major quarters so early tiles land first)
        and allocate this batch's projection output tiles.  With defer_dma,
        the quarter DMAs are returned as closures for just-in-time issue."""
        hsb = hspool.tile([128, NKC, S], F16, tag="hsb", name=f"hsb{b}")
        dmas = []

        def mk(qt):
            def g():
                tsl = bass.ds(qt * 512, 512)
                nc.gpsimd.dma_start(
                    out=hsb[:, :, tsl],
                    in_=hsT[b, :, :, tsl].rearrange("k d c -> d k c"))
            return g

        for qt in range(4):
            if defer_dma:
                dmas.append(mk(qt))
            else:
                mk(qt)()
        qT_b = qpool.tile([128, S], F16, tag="qT", name=f"qT{b}")
        kT_b = kpool.tile([128, S], F16, tag="kT", name=f"kT{b}")
        vTt = vtpool.tile([128, S], F16, tag="vTt", name=f"vTt{b}")
        vn_b = [vnpool.tile([128, NJT, DH + 4], F16, tag=f"vn{hl}",
                            name=f"vn_b{b}h{hl}") for hl in range(HPC)]
        for hl in range(HPC):
            nc.gpsimd.memset(vn_b[hl][:, :, bass.ds(DH, 4)], 1.0)
        qT[b], kT[b], vn[b] = qT_b, kT_b, vn_b
        return (hsb, qT_b, kT_b, vTt, vn_b), dmas

    def proj_closures2(b, tiles, what):
        hsb, qT_b, kT_b, vTt, vn_b = tiles
        ops = []

        def mk_grp(w, sl, ref, kks, start, stop, fin):
            def g():
                if start:
                    ref[0] = pspool.tile([128, 512], F32, tag="ps", name="pj")
                ps = ref[0]
                for i, kk in enumerate(kks):
                    nc.tensor.matmul(ps, lhsT=w_sb[:, w, kk],
                                     rhs=hsb[:, kk, sl],
                                     start=(start and i == 0),
                                     stop=(stop and i == len(kks) - 1))
                if fin is not None:
                    fin(ps)
            return g

        for kind, pt in what:
            if kind == "t":
                for j2 in range(2):
                    def gt(pt=pt, j2=j2):
                        for jt in (pt * 4 + j2 * 2, pt * 4 + j2 * 2 + 1):
                            pst = pspool.tile([128, 128], F16, tag="ps",
                                              name="pst")
                            nc.tensor.transpose(
                                pst, vTt[:, bass.ds(jt * 128, 128)], ident)
                            for hl in range(HPC):
                                nc.vector.tensor_copy(
                                    vn_b[hl][:, jt, bass.ds(0, DH)],
                                    pst[:, bass.ds(hl * DH, DH)])
                    ops.append(gt)
                continue
            w = {"q": 0, "k": 1, "v": 2}[kind]
            sl = bass.ds(pt * 512, 512)
            ref = {}
            if kind == "q":
                def fin(ps, sl=sl, t=qT_b):
                    nc.vector.tensor_scalar_add(t[:, sl], ps, bqc_sb)
            elif kind == "v":
                def fin(ps, sl=sl, t=vTt):
                    nc.vector.tensor_scalar_add(t[:, sl], ps, bvc_sb)
            else:
                def fin(ps, sl=sl, t=kT_b, b=b, pt=pt):
                    nc.tensor.matmul(
                        ps, lhsT=stab_sb,
                        rhs=seg2_sb[:, bass.ds(b * S + pt * 512, 512)],
                        start=False, stop=True)
                    nc.vector.tensor_copy(t[:, sl], ps)
            laststop = kind != "k"      # k closes in fin via the seg matmul
            ops.append(mk_grp(w, sl, ref, [0, 1], True, False, None))
            ops.append(mk_grp(w, sl, ref, [2, 3], False, False, None))
            ops.append(mk_grp(w, sl, ref, [4, 5], False, False, None))
            ops.append(mk_grp(w, sl, ref, [6, 7], False, laststop, fin))
        return ops

    # --- stage B -----------------------------------------------------------
    def rel_in_ap(b, hl, q, ibs):
        src = relT[b, hl] if use_mask else relT[hl]
        return src[bass.ds(q * 2, 2), :, ibs].rearrange("t p i -> p t i")

    def emit_unit(b, ib, fillers, fill_rate, rp, next_fetch=None):
        """One (batch, i-block) unit: 16 jt iterations, software-pipelined.

        rp: erel pair-tile cache keyed (q, hl); shared between the two
        batches of the same i-block (the second batch gets pure cache hits).
        next_fetch: closures prefetching the NEXT i-block's pairs, drained
        one per jt slot from slot 8 (their ring slots free as this unit's
        second-batch muls retire)."""
        ibs = bass.ds(ib * 1024, 1024)

        def fetch(q):
            if q < NJT // 2 and (q, 0) not in rp:
                for hl in range(HPC):
                    r = relpool.tile([128, 2, 1024], F16, tag="rp", name="rp")
                    nc.sync.dma_start(out=r, in_=rel_in_ap(b, hl, q, ibs))
                    rp[(q, hl)] = r

        for q in range(REL_PREFETCH):
            fetch(q)

        pv = [pvpool.tile([DH + 4, 1024], F32, tag="pv", name=f"pv{_hl}")
              for _hl in range(HPC)]
        prev = None

        def emit_pv(pjt, pprob):
            for hl in range(HPC):
                for i2 in range(2):
                    nc.tensor.matmul(
                        pv[hl][:, bass.ds(i2 * 512, 512)],
                        lhsT=vn[b][hl][:, pjt, :],
                        rhs=pprob[hl][:, bass.ds(i2 * 512, 512)],
                        start=(pjt == 0), stop=(pjt == NJT - 1))

        for jt in range(NJT):
            if jt % 2 == 0:
                fetch(jt // 2 + REL_PREFETCH)
            if next_fetch is not None and jt >= 8 and next_fetch:
                next_fetch.pop(0)()
            # QK for this jt (2 x N=512 per head, fp32 PSUM)
            psS = []
            for hl in range(HPC):
                hs_ = bass.ds(hl * DH, DH)
                ps = pspool.tile([128, 1024], F32, tag="ps", name=f"psS{hl}")
                for i2 in range(2):
                    nc.tensor.matmul(
                        ps[:, bass.ds(i2 * 512, 512)],
                        lhsT=kT[b][hs_, bass.ds(jt * 128, 128)],
                        rhs=qT[b][hs_, bass.ds(ib * 1024 + i2 * 512, 512)],
                        start=True, stop=True)
                psS.append(ps)
            # PV for the previous jt (PE stream: right behind this jt's QK)
            if prev is not None:
                emit_pv(*prev)
            # PE slack fillers
            for _ in range(fill_rate):
                if fillers:
                    fillers.pop(0)()
            # exp + erel-multiply for this jt
            probs = []
            for hl in range(HPC):
                col = (b * HPC + hl) * NJT + jt
                eqk = probpool.tile([128, 1024], F16, tag="eqk", name="eqk", bufs=2)
                nc.scalar.activation(eqk, psS[hl],
                                     mybir.ActivationFunctionType.Exp,
                                     bias=r1c[:, col:col + 1], scale=1.0)
                prob = probpool.tile([128, 1024], F16, tag="prob", name="prob")
                nc.vector.tensor_mul(prob, eqk, rp[(jt // 2, hl)][:, jt % 2, :])
                probs.append(prob)
            prev = (jt, probs)
        emit_pv(*prev)
        return pv

    def emit_fin(ib, b, pv):
        ibs = bass.ds(ib * 1024, 1024)
        for hl in range(HPC):
            # evacuate PSUM accumulator promptly so the bank frees up
            pvs = ctxpool.tile([DH + 1, 1024], F32, tag="pvs", name="pvs")
            nc.vector.tensor_copy(pvs, pv[hl][0:DH + 1, :])
            den_dram = scrpool.tile([1, 1024], F32, tag="den_dram")
            rcp_dram = scrpool.tile([1, 1024], F32, tag="rcp_dram")
            nc.sync.dma_start(out=den_dram, in_=pvs[DH:DH + 1, :])
            den_t = denpool.tile([128, 8], F32, tag="den_t")
            nc.sync.dma_start(
                out=den_t,
                in_=bass.AP(den_dram.tensor, den_dram.offset, [[1, 128], [128, 8]]))
            rcp_t = denpool.tile([128, 8], F32, tag="rcp_t")
            nc.vector.reciprocal(rcp_t, den_t)
            nc.sync.dma_start(
                out=bass.AP(rcp_dram.tensor, rcp_dram.offset, [[1, 128], [128, 8]]),
                in_=rcp_t)
            rcpb = rcpbpool.tile([DH, 1024], F32, tag="rcpb")
            nc.sync.dma_start(
                out=rcpb,
                in_=bass.AP(rcp_dram.tensor, rcp_dram.offset, [[0, DH], [1, 1024]]))
            ctxt = ctxpool.tile([DH, 1024], F16, tag="ctxt")
            nc.vector.tensor_mul(ctxt, pvs[0:DH, :], rcpb)
            nc.gpsimd.dma_start(
                out=out[b, bass.ds(hl * DH, DH), ibs], in_=ctxt)

    # --- emission ----------------------------------------------------------
    t0, _ = emit_hsb(0)
    t1, hdma1 = emit_hsb(1, defer_dma=True)
    # warmup: just enough of batch 0 for unit (0,0) to start
    for op in proj_closures2(0, t0, [("q", 0), ("q", 1), ("k", 0),
                                     ("v", 0), ("t", 0)]):
        op()
    emit_r1c()
    f0 = proj_closures2(0, t0, [("k", 1), ("v", 1), ("t", 1),
                                ("k", 2), ("v", 2), ("t", 2),
                                ("k", 3), ("v", 3), ("t", 3)])
    # q2/q3 (needed only by the ib1 units) drip through U2 instead
    f0q = proj_closures2(0, t0, [("q", 2), ("q", 3)])
    f1 = proj_closures2(1, t1, [("q", 0), ("q", 1), ("k", 0),
                                ("v", 0), ("t", 0),
                                ("k", 1), ("v", 1), ("t", 1),
                                ("k", 2), ("v", 2), ("t", 2),
                                ("k", 3), ("v", 3), ("t", 3)])
    f1q = proj_closures2(1, t1, [("q", 2), ("q", 3)])
    # Unit order pairs batches per i-block so each erel pair-tile is loaded
    # once and consumed by both batches (halves erel HBM traffic).  All of
    # proj1 except Q2/Q3 must therefore land inside U1 (crammed, PE-bound);
    # the Q2/Q3 tiles (consumed by the ib1 units) drip through U2.
    # f1 layout: q0(4) q1(4) k0(4) v0(4) t0(2) | k1,v1,t1 k2,v2,t2 k3,v3,t3.
    # U2 = (b1, ib0) consumes ALL of kT[b1]/vn[b1] (j spans the full seq), so
    # every k/v/t piece must land in U1 or stay ahead of U2's jt consumption;
    # only the Q2/Q3 halves (i-block 1) can wait until U2/U3.
    f1a, f1bc = f1[:18], f1[18:]
    fill_u1 = [hdma1[0], hdma1[1]] + f0 + f1a + [hdma1[2], hdma1[3]]
    fill_u2 = f1bc + f0q
    fill_u3 = f1q

    def mk_fetch(b, ib, rp):
        ops = []
        ibs = bass.ds(ib * 1024, 1024)
        for q in range(NJT // 2):
            def g(q=q):
                if (q, 0) in rp:
                    return
                for hl in range(HPC):
                    r = relpool.tile([128, 2, 1024], F16, tag="rp", name="rp")
                    nc.sync.dma_start(out=r, in_=rel_in_ap(b, hl, q, ibs))
                    rp[(q, hl)] = r
            ops.append(g)
        return ops

    rp0, rp1 = {}, {}
    nf1 = mk_fetch(0, 1, rp1)
    pv = emit_unit(0, 0, fill_u1, FILL_RATES[0], rp0)
    emit_fin(0, 0, pv)
    pv = emit_unit(1, 0, fill_u2, FILL_RATES[1], rp0, next_fetch=nf1)
    emit_fin(0, 1, pv)
    pv = emit_unit(0, 1, fill_u3, FILL_RATES[2], rp1)
    emit_fin(1, 0, pv)
    pv = emit_unit(1, 1, fill_u3, FILL_RATES[3], rp1)
    for lst in (fill_u1, fill_u2, fill_u3):
        while lst:
            lst.pop(0)()
    emit_fin(1, 1, pv)


def build_nc(use_mask=False, n_reps=1, opts=None):
    nc = bacc.Bacc("TRN2", target_bir_lowering=False, debug=False,
                   num_devices=N_CORES)
    hsT = nc.declare_dram_parameter("hsT", [B, NKC, 128, S], F16, isOutput=False).ap()
    wT = nc.declare_dram_parameter("wT", [3, NKC, 128, 128], F16, isOutput=False).ap()
    rel_shape = [B, HPC, NJT, 128, S] if use_mask else [HPC, NJT, 128, S]
    relT = nc.declare_dram_parameter("relT", rel_shape, F16, isOutput=False).ap()
    seg2 = nc.declare_dram_parameter("seg2", [B, 2, S], F16, isOutput=False).ap()
    stab = nc.declare_dram_parameter("stab", [2, 128], F16, isOutput=False).ap()
    cpkd = nc.declare_dram_parameter("cpkd", [128, 2 + B * NJT], F32, isOutput=False).ap()
    crowd = nc.declare_dram_parameter("crowd", [1, 3 * 128], F32, isOutput=False).ap()
    out = nc.declare_dram_parameter("out", [B, 128, S], F16, isOutput=True).ap()
    aps = (hsT, wT, relT, seg2, stab, cpkd, crowd, out)

    with tile.TileContext(nc) as tc, ExitStack() as ctx:
        pools = (
            ctx.enter_context(tc.tile_pool(name="const", bufs=1)),
            ctx.enter_context(tc.tile_pool(name="hspool", bufs=B)),
            ctx.enter_context(tc.tile_pool(name="qpool", bufs=B)),
            ctx.enter_context(tc.tile_pool(name="kpool", bufs=B)),
            ctx.enter_context(tc.tile_pool(name="vtpool", bufs=1)),
            ctx.enter_context(tc.tile_pool(name="vnpool", bufs=B)),
            ctx.enter_context(tc.tile_pool(name="relpool", bufs=16)),
            ctx.enter_context(tc.tile_pool(name="probpool", bufs=3)),
            ctx.enter_context(tc.tile_pool(name="pspool", bufs=2, space="PSUM")),
            ctx.enter_context(tc.tile_pool(name="pvpool", bufs=2, space="PSUM")),
            ctx.enter_context(tc.tile_pool(name="denpool", bufs=4)),
            ctx.enter_context(tc.tile_pool(name="rcpbpool", bufs=2)),
            ctx.enter_context(tc.tile_pool(name="ctxpool", bufs=2)),
            ctx.enter_context(tc.tile_pool(name="scrpool", bufs=4, space="DRAM")),
        )
        if n_reps == 1:
            emit_body(nc, tc, ctx, pools, aps, use_mask, opts)
        else:
            hint = (mybir.EngineType.PE, mybir.EngineType.DVE,
                    mybir.EngineType.Activation, mybir.EngineType.SP,
                    mybir.EngineType.Pool)
            with tc.For_i(0, n_reps, 1, hint_engines=hint):
                emit_body(nc, tc, ctx, pools, aps, use_mask, opts)
    nc.compile()
    return nc


# ---------------------------------------------------------------------------
# host side
# ---------------------------------------------------------------------------

def prep_in_maps(hidden_states, attention_mask, rel_pos, seg_ids,
                 Wq, bq, Wk, Wv, bv, seg_table, b_q_s, use_mask):
    hs = np.asarray(hidden_states, np.float32)
    hsT = np.ascontiguousarray(hs.transpose(0, 2, 1)).astype(np.float16)
    hsT = hsT.reshape(B, NKC, 128, S)
    seg = np.asarray(seg_ids).astype(np.float32)          # [B, S]
    seg2 = np.stack([1.0 - seg, seg], axis=1).astype(np.float16)  # [B,2,S]
    segc = np.ascontiguousarray(
        seg.reshape(B, NJT, 128).transpose(0, 2, 1)).astype(np.float32)
    rel = np.asarray(rel_pos, np.float32)[0]              # [H, S, S]
    relT_all = np.ascontiguousarray(rel.transpose(0, 2, 1))    # [H, S(j), S(i)]
    if use_mask:
        mask = np.asarray(attention_mask, np.float32)[:, 0]    # [B, S, S]
        maskT = mask.transpose(0, 2, 1)                        # [B, S(j), S(i)]
        erelT_all = np.exp(relT_all[None, :, :, :] + maskT[:, None, :, :])
        erelT_all = erelT_all.astype(np.float16).reshape(B, H, NJT, 128, S)
    else:
        erelT_all = np.exp(relT_all).astype(np.float16).reshape(H, NJT, 128, S)
    Wq = np.asarray(Wq, np.float32); Wk = np.asarray(Wk, np.float32)
    Wv = np.asarray(Wv, np.float32)
    seg_table = np.asarray(seg_table, np.float32)
    b_q_s = np.asarray(b_q_s, np.float32)
    bq = np.asarray(bq, np.float32); bv = np.asarray(bv, np.float32)

    in_maps = []
    for c in range(N_CORES):
        hc = slice(c * HPC * DH, (c + 1) * HPC * DH)      # 128 head-columns
        wTc = np.stack([
            np.ascontiguousarray(Wq[hc].T),
            np.ascontiguousarray(Wk[hc].T),
            np.ascontiguousarray(Wv[hc].T),
        ]).astype(np.float16).reshape(3, NKC, 128, 128)
        if use_mask:
            relc = np.ascontiguousarray(erelT_all[:, c * HPC:(c + 1) * HPC])
        else:
            relc = np.ascontiguousarray(erelT_all[c * HPC:(c + 1) * HPC])
        cpkd = np.empty((128, 2 + B * NJT), np.float32)
        cpkd[:, 0] = bq[hc]
        cpkd[:, 1] = bv[hc]
        cpkd[:, 2:] = segc.transpose(1, 0, 2).reshape(128, B * NJT)
        crowd = np.concatenate([
            seg_table[0, hc], seg_table[1, hc],
            b_q_s[0, c * HPC:(c + 1) * HPC, 0].reshape(128),
        ]).reshape(1, 384).astype(np.float32)
        m = {
            "hsT": hsT,
            "wT": wTc,
            "relT": relc,
            "seg2": seg2,
            "stab": seg_table[:, hc].astype(np.float16),
            "cpkd": cpkd,
            "crowd": crowd,
        }
        in_maps.append(m)
    return in_maps


def assemble_output(results):
    out = np.empty((B, S, D), np.float32)
    for c in range(N_CORES):
        ctxT = results[c]["out"]                          # [B, 128, S] f16
        hc = slice(c * HPC * DH, (c + 1) * HPC * DH)
        out[:, :, hc] = ctxT.transpose(0, 2, 1).astype(np.float32)
    return out


_CACHED = {}


def kernel(**inputs):
    use_mask = bool(np.any(np.asarray(inputs["attention_mask"])))
    key = ("nc", use_mask)
    if key not in _CACHED:
        _CACHED[key] = build_nc(use_mask=use_mask)
    nc = _CACHED[key]
    in_maps = prep_in_maps(use_mask=use_mask, **inputs)
    res = run_bass_kernel_spmd(nc, in_maps, list(range(N_CORES)))
    return assemble_output(res.results)


# revision 15
# speedup vs baseline: 1.1198x; 1.1198x over previous
"""BertSelfAttention (disentangled seg-bias variant) on 8 Trainium2 NeuronCores.

Sharding: tensor-parallel over heads (2 heads per core); both batches per core.

Math per (b, h):
  q = hs @ Wq.T + bq ; k = hs @ Wk.T ; v = hs @ Wv.T + bv
  k' = scale*k + seg_rep          (seg folded into K-projection PSUM via a
                                   rank-2 matmul: seg_rep = t0*(1-s) + t1*s)
  scoresT[j,i] = sum_c k'[c,j] q[c,i]            (j on partitions)
  r1[j] = b_q_s . seg_rep[j]  -> per-partition bias of the exp
  probsT = exp(scoresT + r1[j]) * erelT          (erel = exp(rel) on host)
  ctxT[c,i] = sum_j v[j,c] probsT[j,i] ; denom via ones-columns in the same MM
  out = ctxT / denom

Schedule: stage B is 4 units (b,ib), 16 j-tiles each, software-pipelined so
the PE stream per j-tile is [QK(jt), PV(jt-1), fillers]; exp (ACT) and the
erel multiply (DVE 2x fp16) chase one j-tile behind.  All projection work
beyond the minimal warmup (Q pt0-1, K pt0, V pt0 + transposes of batch 0) is
drip-fed into PE slack as "filler" closures, ordered so each piece completes
just ahead of its consumer.
"""

import numpy as np
from contextlib import ExitStack

import concourse.bass as bass
import concourse.bacc as bacc
import concourse.mybir as mybir
import concourse.tile as tile
from concourse.bass_utils import run_bass_kernel_spmd
from concourse.masks import make_identity

B, S, D, H = 2, 2048, 1024, 16
DH = D // H                      # 64
N_CORES = 8
HPC = H // N_CORES               # heads per core = 2
NKC = D // 128                   # contraction chunks = 8
NPT = S // 512                   # 512-wide position tiles = 4
NJT = S // 128                   # 128-wide j tiles = 16
NIB = S // 1024                  # 1024-wide i blocks = 2
SCALE = 1.0 / np.sqrt(DH)        # 0.125, exact in fp16

F32 = mybir.dt.float32
F16 = mybir.dt.float16

REL_PREFETCH = 5                 # erel pair-loads emitted ahead
FILL_RATES = (5, 3, 1, 1)        # filler closures per jt slot, per unit


def emit_body(nc, tc, ctx, pools, aps, use_mask, opts=None):
    opts = opts or {}
    (const, hspool, qpool, kpool, vtpool, vnpool, relpool, probpool,
     pspool, pvpool, denpool, rcpbpool, ctxpool, scrpool) = pools
    hsT, wT, relT, seg2, stab, cpkd, crowd, out = aps

    w_sb = const.tile([128, 3, NKC, 128], F16, tag="w_sb")
    nc.sync.dma_start(out=w_sb, in_=wT.rearrange("p k d c -> d p k c"))
    # fold softmax scale into Wk (0.125 is exact in fp16)
    nc.vector.tensor_scalar_mul(w_sb[:, 1], w_sb[:, 1], SCALE)

    # packed fp32 per-partition consts: col0 bqc, col1 bvc, col2.. segc
    cpk = const.tile([128, 2 + B * NJT], F32, tag="cpk")
    nc.sync.dma_start(out=cpk, in_=cpkd)
    bqc_sb = cpk[:, 0:1]
    bvc_sb = cpk[:, 1:2]
    segc_sb = cpk[:, 2:2 + B * NJT]
    # packed fp32 row consts along the free dim: t0 | t1 | bqs
    crow = const.tile([1, 3 * 128], F32, tag="crow")
    nc.sync.dma_start(out=crow, in_=crowd)
    t0f = crow[0:1, 0:128]
    t1f = crow[0:1, 128:256]
    bqs_sb = crow[0:1, 256:384]

    stab_sb = const.tile([2, 128], F16, tag="stab_sb")
    nc.sync.dma_start(out=stab_sb, in_=stab)
    seg2_sb = const.tile([2, B * S], F16, tag="seg2_sb")
    nc.sync.dma_start(out=seg2_sb, in_=seg2.rearrange("b r s -> r b s"))

    ident = const.tile([128, 128], F16, tag="ident")
    make_identity(nc, ident)

    # --- r1 (b_q_s . seg_rep) per-partition bias columns -------------------
    # gamma0_h = sum_{c in head h} bqs[c]*t0[c]; gamma1_h likewise with t1.
    # r1[j] = gamma0_h + (gamma1_h - gamma0_h) * s_j
    prod = const.tile([1, 128], F32, tag="prod")
    g_row = const.tile([1, 4], F32, tag="g_row")   # [g0_h0, g0_h1, g1_h0, g1_h1]
    b_row = const.tile([1, 4], F32, tag="b_row")   # [d_h0, d_h1, g0_h0, g0_h1]
    ones1 = const.tile([1, 128], F32, tag="ones1")
    nc.vector.memset(ones1, 1.0)
    nc.vector.tensor_mul(prod, bqs_sb, t0f)
    nc.vector.tensor_reduce(g_row[0:1, 0:1], prod[0:1, 0:64],
                            axis=mybir.AxisListType.X, op=mybir.AluOpType.add)
    nc.vector.tensor_reduce(g_row[0:1, 1:2], prod[0:1, 64:128],
                            axis=mybir.AxisListType.X, op=mybir.AluOpType.add)
    nc.vector.tensor_mul(prod, bqs_sb, t1f)
    nc.vector.tensor_reduce(g_row[0:1, 2:3], prod[0:1, 0:64],
                            axis=mybir.AxisListType.X, op=mybir.AluOpType.add)
    nc.vector.tensor_reduce(g_row[0:1, 3:4], prod[0:1, 64:128],
                            axis=mybir.AxisListType.X, op=mybir.AluOpType.add)
    # delta_h = g1_h - g0_h in slots 0:2, g0_h in slots 2:4
    nc.vector.tensor_sub(b_row[0:1, 0:2], g_row[0:1, 2:4], g_row[0:1, 0:2])
    nc.vector.tensor_copy(b_row[0:1, 2:4], g_row[0:1, 0:2])
    # broadcast the 4 values to all 128 partitions via a K=1 matmul.
    # Emitted via closure AFTER the warmup projections so the tiny psB
    # matmul does not sit at the head of the PE stream gating everything
    # on the (late-arriving) const DMAs.
    r1c = const.tile([128, B * HPC * NJT], F32, tag="r1c")

    def emit_r1c():
        psB = pspool.tile([128, 4], F32, tag="ps", name="psB")
        nc.tensor.matmul(psB, lhsT=ones1, rhs=b_row, start=True, stop=True)
        bc4 = const.tile([128, 4], F32, tag="bc4")
        nc.vector.tensor_copy(bc4, psB)
        for b in range(B):
            for hl in range(HPC):
                nc.vector.tensor_scalar(
                    out=r1c[:, (b * HPC + hl) * NJT:(b * HPC + hl + 1) * NJT],
                    in0=segc_sb[:, b * NJT:(b + 1) * NJT],
                    scalar1=bc4[:, hl:hl + 1],
                    scalar2=bc4[:, 2 + hl:2 + hl + 1],
                    op0=mybir.AluOpType.mult,
                    op1=mybir.AluOpType.add,
                )

    # --- projection machinery ---------------------------------------------
    qT, kT, vn = [None] * B, [None] * B, [None] * B

    def emit_hsb(b, defer_dma=False):
        """DMA hidden-states (token-major quarters so early tiles land first)
        and allocate this batch's projection output tiles.  With defer_dma,
        the quarter DMAs are returned as closures for just-in-time issue."""
        hsb = hspool.tile([128, NKC, S], F16, tag="hsb", name=f"hsb{b}")
        dmas = []

        def mk(qt):
            def g():
                tsl = bass.ds(qt * 512, 512)
                nc.sync.dma_start(
                    out=hsb[:, :, tsl],
                    in_=hsT[b, :, :, tsl].rearrange("k d c -> d k c"))
            return g

        for qt in range(4):
            if defer_dma:
                dmas.append(mk(qt))
            else:
                mk(qt)()
        qT_b = qpool.tile([128, S], F16, tag="qT", name=f"qT{b}")
        kT_b = kpool.tile([128, S], F16, tag="kT", name=f"kT{b}")
        vTt = vtpool.tile([128, S], F16, tag="vTt", name=f"vTt{b}")
        vn_b = [vnpool.tile([128, NJT, DH + 4], F16, tag=f"vn{hl}",
                            name=f"vn_b{b}h{hl}") for hl in range(HPC)]
        for hl in range(HPC):
            nc.gpsimd.memset(vn_b[hl][:, :, bass.ds(DH, 4)], 1.0)
        qT[b], kT[b], vn[b] = qT_b, kT_b, vn_b
        return (hsb, qT_b, kT_b, vTt, vn_b), dmas

    def proj_closures2(b, tiles, what):
        hsb, qT_b, kT_b, vTt, vn_b = tiles
        ops = []

        def mk_grp(w, sl, ref, kks, start, stop, fin):
            def g():
                if start:
                    ref[0] = pspool.tile([128, 512], F32, tag="ps", name="pj")
                ps = ref[0]
                for i, kk in enumerate(kks):
                    nc.tensor.matmul(ps, lhsT=w_sb[:, w, kk],
                                     rhs=hsb[:, kk, sl],
                                     start=(start and i == 0),
                                     stop=(stop and i == len(kks) - 1))
                if fin is not None:
                    fin(ps)
            return g

        for kind, pt in what:
            if kind == "t":
                for j2 in range(2):
                    def gt(pt=pt, j2=j2):
                        for jt in (pt * 4 + j2 * 2, pt * 4 + j2 * 2 + 1):
                            pst = pspool.tile([128, 128], F16, tag="ps",
                                              name="pst")
                            nc.tensor.transpose(
                                pst, vTt[:, bass.ds(jt * 128, 128)], ident)
                            for hl in range(HPC):
                                nc.vector.tensor_copy(
                                    vn_b[hl][:, jt, bass.ds(0, DH)],
                                    pst[:, bass.ds(hl * DH, DH)])
                    ops.append(gt)
                continue
            w = {"q": 0, "k": 1, "v": 2}[kind]
            sl = bass.ds(pt * 512, 512)
            ref = {}
            if kind == "q":
                def fin(ps, sl=sl, t=qT_b):
                    nc.vector.tensor_scalar_add(t[:, sl], ps, bqc_sb)
            elif kind == "v":
                def fin(ps, sl=sl, t=vTt):
                    nc.vector.tensor_scalar_add(t[:, sl], ps, bvc_sb)
            else:
                def fin(ps, sl=sl, t=kT_b, b=b, pt=pt):
                    nc.tensor.matmul(
                        ps, lhsT=stab_sb,
                        rhs=seg2_sb[:, bass.ds(b * S + pt * 512, 512)],
                        start=False, stop=True)
                    nc.vector.tensor_copy(t[:, sl], ps)
            laststop = kind != "k"      # k closes in fin via the seg matmul
            ops.append(mk_grp(w, sl, ref, [0, 1], True, False, None))
            ops.append(mk_grp(w, sl, ref, [2, 3], False, False, None))
            ops.append(mk_grp(w, sl, ref, [4, 5], False, False, None))
            ops.append(mk_grp(w, sl, ref, [6, 7], False, laststop, fin))
        return ops

    # --- stage B -----------------------------------------------------------
    def rel_in_ap(b, hl, q, ibs):
        src = relT[b, hl] if use_mask else relT[hl]
        return src[bass.ds(q * 2, 2), :, ibs].rearrange("t p i -> p t i")

    def emit_unit(b, ib, fillers, fill_rate, rp, next_fetch=None):
        """One (batch, i-block) unit: 16 jt iterations, software-pipelined.

        rp: erel pair-tile cache keyed (q, hl); shared between the two
        batches of the same i-block (the second batch gets pure cache hits).
        next_fetch: closures prefetching the NEXT i-block's pairs, drained
        one per jt slot from slot 8 (their ring slots free as this unit's
        second-batch muls retire)."""
        ibs = bass.ds(ib * 1024, 1024)

        def fetch(q):
            if q < NJT // 2 and (q, 0) not in rp:
                for hl in range(HPC):
                    r = relpool.tile([128, 2, 1024], F16, tag="rp", name="rp")
                    nc.sync.dma_start(out=r, in_=rel_in_ap(b, hl, q, ibs))
                    rp[(q, hl)] = r

        for q in range(REL_PREFETCH):
            fetch(q)

        pv = [pvpool.tile([DH + 4, 1024], F32, tag="pv", name=f"pv{_hl}")
              for _hl in range(HPC)]
        prev = None

        def emit_pv(pjt, pprob):
            for hl in range(HPC):
                for i2 in range(2):
                    nc.tensor.matmul(
                        pv[hl][:, bass.ds(i2 * 512, 512)],
                        lhsT=vn[b][hl][:, pjt, :],
                        rhs=pprob[hl][:, bass.ds(i2 * 512, 512)],
                        start=(pjt == 0), stop=(pjt == NJT - 1))

        for jt in range(NJT):
            if jt % 2 == 0:
                fetch(jt // 2 + REL_PREFETCH)
            if next_fetch is not None and jt >= 8 and next_fetch:
                next_fetch.pop(0)()
            # QK for this jt (2 x N=512 per head, fp32 PSUM)
            psS = []
            for hl in range(HPC):
                hs_ = bass.ds(hl * DH, DH)
                ps = pspool.tile([128, 1024], F32, tag="ps", name=f"psS{hl}")
                for i2 in range(2):
                    nc.tensor.matmul(
                        ps[:, bass.ds(i2 * 512, 512)],
                        lhsT=kT[b][hs_, bass.ds(jt * 128, 128)],
                        rhs=qT[b][hs_, bass.ds(ib * 1024 + i2 * 512, 512)],
                        start=True, stop=True)
                psS.append(ps)
            # PV for the previous jt (PE stream: right behind this jt's QK)
            if prev is not None:
                emit_pv(*prev)
            # PE slack fillers
            for _ in range(fill_rate):
                if fillers:
                    fillers.pop(0)()
            # exp + erel-multiply for this jt
            probs = []
            for hl in range(HPC):
                col = (b * HPC + hl) * NJT + jt
                eqk = probpool.tile([128, 1024], F16, tag="eqk", name="eqk", bufs=2)
                nc.scalar.activation(eqk, psS[hl],
                                     mybir.ActivationFunctionType.Exp,
                                     bias=r1c[:, col:col + 1], scale=1.0)
                prob = probpool.tile([128, 1024], F16, tag="prob", name="prob")
                nc.vector.tensor_mul(prob, eqk, rp[(jt // 2, hl)][:, jt % 2, :])
                probs.append(prob)
            prev = (jt, probs)
        emit_pv(*prev)
        return pv

    def emit_fin(ib, b, pv):
        ibs = bass.ds(ib * 1024, 1024)
        for hl in range(HPC):
            # evacuate PSUM accumulator promptly so the bank frees up
            pvs = ctxpool.tile([DH + 1, 1024], F32, tag="pvs", name="pvs")
            nc.vector.tensor_copy(pvs, pv[hl][0:DH + 1, :])
            den_dram = scrpool.tile([1, 1024], F32, tag="den_dram")
            rcp_dram = scrpool.tile([1, 1024], F32, tag="rcp_dram")
            nc.sync.dma_start(out=den_dram, in_=pvs[DH:DH + 1, :])
            den_t = denpool.tile([128, 8], F32, tag="den_t")
            nc.sync.dma_start(
                out=den_t,
                in_=bass.AP(den_dram.tensor, den_dram.offset, [[1, 128], [128, 8]]))
            rcp_t = denpool.tile([128, 8], F32, tag="rcp_t")
            nc.vector.reciprocal(rcp_t, den_t)
            nc.sync.dma_start(
                out=bass.AP(rcp_dram.tensor, rcp_dram.offset, [[1, 128], [128, 8]]),
                in_=rcp_t)
            rcpb = rcpbpool.tile([DH, 1024], F32, tag="rcpb")
            nc.sync.dma_start(
                out=rcpb,
                in_=bass.AP(rcp_dram.tensor, rcp_dram.offset, [[0, DH], [1, 1024]]))
            ctxt = ctxpool.tile([DH, 1024], F16, tag="ctxt")
            nc.vector.tensor_mul(ctxt, pvs[0:DH, :], rcpb)
            nc.sync.dma_start(
                out=out[b, bass.ds(hl * DH, DH), ibs], in_=ctxt)

    # --- emission ----------------------------------------------------------
    t0, _ = emit_hsb(0)
    t1, hdma1 = emit_hsb(1, defer_dma=True)
    # warmup: just enough of batch 0 for unit (0,0) to start
    for op in proj_closures2(0, t0, [("q", 0), ("q", 1), ("k", 0),
                                     ("v", 0), ("t", 0)]):
        op()
    emit_r1c()
    f0 = proj_closures2(0, t0, [("k", 1), ("v", 1), ("t", 1),
                                ("k", 2), ("v", 2), ("t", 2),
                                ("k", 3), ("v", 3), ("t", 3)])
    # q2/q3 (needed only by the ib1 units) drip through U2 instead
    f0q = proj_closures2(0, t0, [("q", 2), ("q", 3)])
    f1 = proj_closures2(1, t1, [("q", 0), ("q", 1), ("k", 0),
                                ("v", 0), ("t", 0),
                                ("k", 1), ("v", 1), ("t", 1),
                                ("k", 2), ("v", 2), ("t", 2),
                                ("k", 3), ("v", 3), ("t", 3)])
    f1q = proj_closures2(1, t1, [("q", 2), ("q", 3)])
    # Unit order pairs batches per i-block so each erel pair-tile is loaded
    # once and consumed by both batches (halves erel HBM traffic).  All of
    # proj1 except Q2/Q3 must therefore land inside U1 (crammed, PE-bound);
    # the Q2/Q3 tiles (consumed by the ib1 units) drip through U2.
    # f1 layout: q0(4) q1(4) k0(4) v0(4) t0(2) | k1,v1,t1 k2,v2,t2 k3,v3,t3.
    # U2 = (b1, ib0) consumes ALL of kT[b1]/vn[b1] (j spans the full seq), so
    # every k/v/t piece must land in U1 or stay ahead of U2's jt consumption;
    # only the Q2/Q3 halves (i-block 1) can wait until U2/U3.
    f1a, f1bc = f1[:18], f1[18:]
    fill_u1 = [hdma1[0], hdma1[1]] + f0 + f1a + [hdma1[2], hdma1[3]]
    fill_u2 = f1bc + f0q
    fill_u3 = f1q

    def mk_fetch(b, ib, rp):
        ops = []
        ibs = bass.ds(ib * 1024, 1024)
        for q in range(NJT // 2):
            def g(q=q):
                if (q, 0) in rp:
                    return
                for hl in range(HPC):
                    r = relpool.tile([128, 2, 1024], F16, tag="rp", name="rp")
                    nc.sync.dma_start(out=r, in_=rel_in_ap(b, hl, q, ibs))
                    rp[(q, hl)] = r
            ops.append(g)
        return ops

    rp0, rp1 = {}, {}
    nf1 = mk_fetch(0, 1, rp1)
    pv = emit_unit(0, 0, fill_u1, FILL_RATES[0], rp0)
    emit_fin(0, 0, pv)
    pv = emit_unit(1, 0, fill_u2, FILL_RATES[1], rp0, next_fetch=nf1)
    emit_fin(0, 1, pv)
    pv = emit_unit(0, 1, fill_u3, FILL_RATES[2], rp1)
    emit_fin(1, 0, pv)
    pv = emit_unit(1, 1, fill_u3, FILL_RATES[3], rp1)
    for lst in (fill_u1, fill_u2, fill_u3):
        while lst:
            lst.pop(0)()
    emit_fin(1, 1, pv)


def build_nc(use_mask=False, n_reps=1, opts=None):
    nc = bacc.Bacc("TRN2", target_bir_lowering=False, debug=False,
                   num_devices=N_CORES)
    hsT = nc.declare_dram_parameter("hsT", [B, NKC, 128, S], F16, isOutput=False).ap()
    wT = nc.declare_dram_parameter("wT", [3, NKC, 128, 128], F16, isOutput=False).ap()
    rel_shape = [B, HPC, NJT, 128, S] if use_mask else [HPC, NJT, 128, S]
    relT = nc.declare_dram_parameter("relT", rel_shape, F16, isOutput=False).ap()
    seg2 = nc.declare_dram_parameter("seg2", [B, 2, S], F16, isOutput=False).ap()
    stab = nc.declare_dram_parameter("stab", [2, 128], F16, isOutput=False).ap()
    cpkd = nc.declare_dram_parameter("cpkd", [128, 2 + B * NJT], F32, isOutput=False).ap()
    crowd = nc.declare_dram_parameter("crowd", [1, 3 * 128], F32, isOutput=False).ap()
    out = nc.declare_dram_parameter("out", [B, 128, S], F16, isOutput=True).ap()
    aps = (hsT, wT, relT, seg2, stab, cpkd, crowd, out)

    with tile.TileContext(nc) as tc, ExitStack() as ctx:
        pools = (
            ctx.enter_context(tc.tile_pool(name="const", bufs=1)),
            ctx.enter_context(tc.tile_pool(name="hspool", bufs=B)),
            ctx.enter_context(tc.tile_pool(name="qpool", bufs=B)),
            ctx.enter_context(tc.tile_pool(name="kpool", bufs=B)),
            ctx.enter_context(tc.tile_pool(name="vtpool", bufs=1)),
            ctx.enter_context(tc.tile_pool(name="vnpool", bufs=B)),
            ctx.enter_context(tc.tile_pool(name="relpool", bufs=16)),
            ctx.enter_context(tc.tile_pool(name="probpool", bufs=3)),
            ctx.enter_context(tc.tile_pool(name="pspool", bufs=2, space="PSUM")),
            ctx.enter_context(tc.tile_pool(name="pvpool", bufs=2, space="PSUM")),
            ctx.enter_context(tc.tile_pool(name="denpool", bufs=4)),
            ctx.enter_context(tc.tile_pool(name="rcpbpool", bufs=2)),
            ctx.enter_context(tc.tile_pool(name="ctxpool", bufs=2)),
            ctx.enter_context(tc.tile_pool(name="scrpool", bufs=4, space="DRAM")),
        )
        if n_reps == 1:
            for _ in range((opts or {}).get("unroll", 1)):
                emit_body(nc, tc, ctx, pools, aps, use_mask, opts)
        else:
            hint = (mybir.EngineType.PE, mybir.EngineType.DVE,
                    mybir.EngineType.Activation, mybir.EngineType.SP,
                    mybir.EngineType.Pool)
            with tc.For_i(0, n_reps, 1, hint_engines=hint):
                emit_body(nc, tc, ctx, pools, aps, use_mask, opts)
    nc.compile()
    return nc


# ---------------------------------------------------------------------------
# host side
# ---------------------------------------------------------------------------

def prep_in_maps(hidden_states, attention_mask, rel_pos, seg_ids,
                 Wq, bq, Wk, Wv, bv, seg_table, b_q_s, use_mask):
    hs = np.asarray(hidden_states, np.float32)
    hsT = np.ascontiguousarray(hs.transpose(0, 2, 1)).astype(np.float16)
    hsT = hsT.reshape(B, NKC, 128, S)
    seg = np.asarray(seg_ids).astype(np.float32)          # [B, S]
    seg2 = np.stack([1.0 - seg, seg], axis=1).astype(np.float16)  # [B,2,S]
    segc = np.ascontiguousarray(
        seg.reshape(B, NJT, 128).transpose(0, 2, 1)).astype(np.float32)
    rel = np.asarray(rel_pos, np.float32)[0]              # [H, S, S]
    relT_all = np.ascontiguousarray(rel.transpose(0, 2, 1))    # [H, S(j), S(i)]
    if use_mask:
        mask = np.asarray(attention_mask, np.float32)[:, 0]    # [B, S, S]
        maskT = mask.transpose(0, 2, 1)                        # [B, S(j), S(i)]
        erelT_all = np.exp(relT_all[None, :, :, :] + maskT[:, None, :, :])
        erelT_all = erelT_all.astype(np.float16).reshape(B, H, NJT, 128, S)
    else:
        erelT_all = np.exp(relT_all).astype(np.float16).reshape(H, NJT, 128, S)
    Wq = np.asarray(Wq, np.float32); Wk = np.asarray(Wk, np.float32)
    Wv = np.asarray(Wv, np.float32)
    seg_table = np.asarray(seg_table, np.float32)
    b_q_s = np.asarray(b_q_s, np.float32)
    bq = np.asarray(bq, np.float32); bv = np.asarray(bv, np.float32)

    in_maps = []
    for c in range(N_CORES):
        hc = slice(c * HPC * DH, (c + 1) * HPC * DH)      # 128 head-columns
        wTc = np.stack([
            np.ascontiguousarray(Wq[hc].T),
            np.ascontiguousarray(Wk[hc].T),
            np.ascontiguousarray(Wv[hc].T),
        ]).astype(np.float16).reshape(3, NKC, 128, 128)
        if use_mask:
            relc = np.ascontiguousarray(erelT_all[:, c * HPC:(c + 1) * HPC])
        else:
            relc = np.ascontiguousarray(erelT_all[c * HPC:(c + 1) * HPC])
        cpkd = np.empty((128, 2 + B * NJT), np.float32)
        cpkd[:, 0] = bq[hc]
        cpkd[:, 1] = bv[hc]
        cpkd[:, 2:] = segc.transpose(1, 0, 2).reshape(128, B * NJT)
        crowd = np.concatenate([
            seg_table[0, hc], seg_table[1, hc],
            b_q_s[0, c * HPC:(c + 1) * HPC, 0].reshape(128),
        ]).reshape(1, 384).astype(np.float32)
        m = {
            "hsT": hsT,
            "wT": wTc,
            "relT": relc,
            "seg2": seg2,
            "stab": seg_table[:, hc].astype(np.float16),
            "cpkd": cpkd,
            "crowd": crowd,
        }
        in_maps.append(m)
    return in_maps


def assemble_output(results):
    out = np.empty((B, S, D), np.float32)
    for c in range(N_CORES):
        ctxT = results[c]["out"]                          # [B, 128, S] f16
        hc = slice(c * HPC * DH, (c + 1) * HPC * DH)
        out[:, :, hc] = ctxT.transpose(0, 2, 1).astype(np.float32)
    return out


_CACHED = {}


def kernel(**inputs):
    use_mask = bool(np.any(np.asarray(inputs["attention_mask"])))
    key = ("nc", use_mask)
    if key not in _CACHED:
        _CACHED[key] = build_nc(use_mask=use_mask)
    nc = _CACHED[key]
    in_maps = prep_in_maps(use_mask=use_mask, **inputs)
    res = run_bass_kernel_spmd(nc, in_maps, list(range(N_CORES)))
    return assemble_output(res.results)
